# revision 45
# baseline (speedup 1.0000x reference)
"""Trainium2 Bass kernel for nn_DilatedKnnGraph (dilated knn edge list).

Problem: x is (65536, 256) fp32 = 64 strokes x 1024 points x 256 dims.
Per stroke: pairwise sq-distance matrix (1024x1024), top-18 neighbors per
point (k=9, dilation=2), edge list (2, S*L*18) sliced [::2] ->
output (2, 589824) int32: row0 = neighbor indices at even ranks
0,2,...,16; row1 = center index repeated 9x.

Sharding: data parallel over strokes; each of the 8 cores handles 8
strokes and emits its contiguous (2, 73728) slice of the edge list.

Per-core device algorithm (per stroke s, row-tile t of 128 points):
 - XT (256x1024 feature-major) via PE transposes; sq via ACT Square+accum,
   PE-transposed into a free-dim row.  Stroke prep is emitted one x-tile
   per main-loop tile so its ACT work never bursts.
 - brow = -0.5*sq + SHIFT (two half-row ACT ops through a 1-bank psum
   slot).  V0 psum (128,1024) accumulates on PE: 2 fp32r K=128 Gram
   matmuls (fp32r runs 1 cycle/row at N>=256 vs 4 for fp32; operand
   tiles are allocated float32r so their ACT producers pre-round, which
   the BIR verifier requires) + K=1 fp32r bias matmul + bf16 (-1e6*I)@I
   on the self-distance diagonal block.  Row ordering of V0 equals
   ordering of -distance^2.
 - Top-16 per row in 3 DVE scans + 1 Pool scan + 2 ACT copies:
   ACT copies V0 psum->sbuf; DVE max8 -> ranks 0-7; Pool tensor_scalar
   msk = (v0 >= rank7) (is_ge is the one per-partition-scalar opcode the
   Pool engine accepts -- mod / scalar_tensor_tensor fail the ISA engine
   check); PE folds (-1e6*I) @ msk into the still-open psum group (kept
   scores stay bit-exact); ACT copies the zapped psum -> v1; DVE max8 on
   v1 -> ranks 8-15.  Both rank-rows land adjacently in one (128,16)
   tile so one DVE max_index consumes the odd ranks 1,3,..,15 as a
   strided view and writes the 8 neighbor column ids per tile straight
   into the uint32 staging tile.  The zap chain is consumed via a
   two-stage one-tile-deferred pipeline (3 psum slots) so the in-order
   DVE/ACT/PE streams never block on the freshest tile.
 - Centers (staging col 0 of each 9-block) come from a host iota
   constant, one small strided copy per stroke; one Pool tensor_scalar
   per stroke adds the per-stroke global base (host-provided f32 column,
   exact below 2^24) before the staging DMA.
 - row1 (center repeated 9x) is a host-precomputed iota constant DMA'd
   dram->dram.

TimelineSim: 238.3us/core vs 388.7us for the previous kernel (measured
459.6us on HW); DVE busy 91.0% at its 3-scan floor (216.9us).  XT merges
are pairwise (one (128,512) ps_t slot, one ACT copy per two x-tiles), x
loads come as (128,512) pair-tiles (one strided DMA per two x-tiles,
halving the ~625ns/DMA HWDGE serialization), and the startup DMA order
interleaves the transpose/diag constants into the x stream so the first
tile's chain pipelines behind it.
"""

import ml_dtypes
import numpy as np

import bass_rust
import concourse.bass as bass
import concourse.mybir as mybir
import concourse.tile as tile_mod
from concourse.alu_op_type import AluOpType
from concourse.bass_utils import run_bass_kernel_spmd
from concourse.tile import TileContext
from concourse.tile_rust import add_dep_helper
from concourse.vector_clock import ScopedClock

S, L, D = 64, 1024, 256
N_CORES = 8
S_PER_CORE = S // N_CORES          # 8 strokes per core
KOUT = 9                           # surviving neighbors per point
PTS_PER_CORE = S_PER_CORE * L      # 8192
COLS_PER_CORE = PTS_PER_CORE * KOUT  # 73728
NEG_BIG = -1.0e6                   # self-distance / round-1 winner zap
SHIFT = 1000.0                     # bias shift making non-self scores > 0
F32 = mybir.dt.float32
F32R = mybir.dt.float32r
U32 = mybir.dt.uint32
BF16 = mybir.dt.bfloat16

# ---------------------------------------------------------------------------
# Workaround: the walrus build in this container rejects instructions that
# carry more than a couple of semaphore waits ("Too many sync wait
# commands").  (1) replace TileContext's final Drain (which carries the whole
# global clock) with single-wait SP EventSemaphore nops; (2) post-pass that
# hoists excess waits from any instruction onto same-engine nops.
# ---------------------------------------------------------------------------
_MAX_WAITS = 1
_wsplit_ctr = [0]


def _mk_wait_carrier(engine, waits):
    _wsplit_ctr[0] += 1
    nop = bass_rust.InstEventSemaphore(
        name=f"I-wsplit-{_wsplit_ctr[0]}", ins=[], outs=[]
    )
    nop.engine = engine
    nop.sync_info = bass_rust.SyncInfo(on_wait=list(waits), on_update=[])
    return nop


def _patched_drain_and_barrier(self, tick_clock, wait_clock):
    nc = self.nc
    collector = nc.sync.nop()
    wait_clock.add_sem_waits(
        collector.ins, ScopedClock({None: tick_clock.global_clock})
    )
    si = collector.ins.sync_info
    waits = list(si.on_wait) if (si and si.on_wait) else []
    if len(waits) > _MAX_WAITS:
        si.on_wait = waits[:_MAX_WAITS]
        rest = waits[_MAX_WAITS:]
        for i in range(0, len(rest), _MAX_WAITS):
            chunk = rest[i : i + _MAX_WAITS]
            nop = nc.sync.nop()
            nsi = nop.ins.sync_info
            if nsi is None:
                nop.ins.sync_info = bass_rust.SyncInfo(on_wait=chunk, on_update=[])
            else:
                nsi.on_wait = list(nsi.on_wait or []) + chunk
    nc.sync.drain()

    nc.all_engine_barrier()
    assert self.sems is not None
    popped = nc._tile_sem_poison_stack.pop()
    assert popped is self._sem_poison
    nc.clear_and_free_semaphores(list(self.sems.allocated().values()))
    nc.all_engine_barrier()


tile_mod.TileContext._drain_and_barrier = _patched_drain_and_barrier


# ---------------------------------------------------------------------------
# Workaround 2: bass2jax.run_bass_via_pjrt converts the shard_map output to
# numpy via np.asarray on the GLOBAL sharded array, which makes jax compile a
# cross-device gather HLO through the full neuronx-cc pipeline — and that
# gather module fails codegen here.  Replace with a version that fetches each
# device's shard directly (plain D2H copies, no gather program).
# ---------------------------------------------------------------------------
def _install_pjrt_patch():
    import jax
    from jax.sharding import Mesh, PartitionSpec
    try:
        from jax.experimental.shard_map import shard_map
    except ImportError:
        from jax.shard_map import shard_map  # newer jax
    from concourse import bass2jax as b2j

    if getattr(b2j, "_knn_patch_installed", False):
        return

    def run_bass_via_pjrt(nc, in_maps, n_cores):
        b2j.install_neuronx_cc_hook()
        assert nc.dbg_addr is None, "debug not supported in patched runner"
        partition_name = (
            nc.partition_id_tensor.name if nc.partition_id_tensor else None
        )

        in_names, out_names, out_avals, zero_outs = [], [], [], []
        for alloc in nc.m.functions[0].allocations:
            if not isinstance(alloc, mybir.MemoryLocationSet):
                continue
            name = alloc.memorylocations[0].name
            if alloc.kind == "ExternalInput":
                if name != partition_name:
                    in_names.append(name)
            elif alloc.kind == "ExternalOutput":
                shape = list(alloc.tensor_shape)
                np_dtype = mybir.dt.np(alloc.dtype)
                out_names.append(name)
                out_avals.append(
                    jax.core.ShapedArray(tuple(shape), np_dtype)
                )
                zero_outs.append(np.zeros(shape, np_dtype))

        n_params = len(in_names)
        n_outs = len(out_avals)
        in_names.extend(out_names)
        if partition_name is not None:
            in_names.append(partition_name)

        donate = tuple(range(n_params, n_params + n_outs))

        def _body(*args):
            operands = list(args)
            if partition_name is not None:
                operands.append(b2j.partition_id_tensor())
            outs = b2j._bass_exec_p.bind(
                *operands,
                out_avals=tuple(out_avals),
                in_names=tuple(in_names),
                out_names=tuple(out_names),
                lowering_input_output_aliases=(),
                sim_require_finite=True,
                sim_require_nnan=True,
                nc=nc,
            )
            return tuple(outs)

        devices = jax.devices()[:n_cores]
        assert len(devices) == n_cores
        mesh = Mesh(np.asarray(devices), ("core",))
        in_specs = (PartitionSpec("core"),) * (n_params + n_outs)
        out_specs = (PartitionSpec("core"),) * len(out_names)
        sharded = jax.jit(
            shard_map(
                _body,
                mesh=mesh,
                in_specs=in_specs,
                out_specs=out_specs,
                check_rep=False,
            ),
            donate_argnums=donate,
            keep_unused=True,
        )
        per_core = [
            [np.asarray(m[name]) for name in in_names[:n_params]] for m in in_maps
        ]
        concat_in = [
            np.concatenate([per_core[c][i] for c in range(n_cores)], axis=0)
            for i in range(n_params)
        ]
        concat_zeros = [
            np.zeros((n_cores * z.shape[0], *z.shape[1:]), z.dtype)
            for z in zero_outs
        ]
        out_arrs = sharded(*concat_in, *concat_zeros)

        results = [dict() for _ in range(n_cores)]
        for i, name in enumerate(out_names):
            arr = out_arrs[i]
            shards = sorted(
                arr.addressable_shards, key=lambda s: s.index[0].start or 0
            )
            assert len(shards) == n_cores
            for c, sh in enumerate(shards):
                results[c][name] = np.asarray(sh.data)
        return results

    b2j.run_bass_via_pjrt = run_bass_via_pjrt
    b2j._knn_patch_installed = True


_install_pjrt_patch()


def _split_sync_waits(nc, max_waits=_MAX_WAITS):
    for f in nc.m.functions:
        for bb in f.blocks:
            changed = False
            new_insts = []
            for inst in bb.instructions:
                si = inst.sync_info
                waits = list(si.on_wait) if (si and si.on_wait) else []
                if len(waits) > max_waits:
                    keep = waits[-max_waits:]
                    extra = waits[:-max_waits]
                    for j in range(0, len(extra), max_waits):
                        new_insts.append(
                            _mk_wait_carrier(inst.engine, extra[j : j + max_waits])
                        )
                    si.on_wait = keep
                    changed = True
                new_insts.append(inst)
            if changed:
                bb.instructions = new_insts


# ---------------------------------------------------------------------------
# Bass program (identical on all 8 cores; per-core data via in_maps)
# ---------------------------------------------------------------------------
def _build_program(n_strokes=S_PER_CORE, split_waits=True, mode="full",
                   gram_dtype="f32r"):
    nc = bass.Bass(target_bir_lowering=False, trn_type="TRN2")
    pts = n_strokes * L
    cols = pts * KOUT
    x_in = nc.dram_tensor("x_shard", [pts, D], F32, kind="ExternalInput")
    row1_in = nc.dram_tensor("row1_const", [cols], U32, kind="ExternalInput")
    base_in = nc.dram_tensor("base_cols", [128, n_strokes], F32, kind="ExternalInput")
    ident_in = nc.dram_tensor("ident_c", [128, 128], F32, kind="ExternalInput")
    identb_in = nc.dram_tensor("identb_c", [128, 128], BF16, kind="ExternalInput")
    negidentb_in = nc.dram_tensor("negidentb_c", [128, 128], BF16, kind="ExternalInput")
    ones1_in = nc.dram_tensor("ones1_c", [1, 128], F32, kind="ExternalInput")
    c128_in = nc.dram_tensor("c128_c", [128, 8], U32, kind="ExternalInput")
    edges = nc.dram_tensor("edges", [2, cols], U32, kind="ExternalOutput")

    # fp32r inputs must be written pre-rounded by their producer (BIR
    # verifier rule), so the Gram operand tiles are allocated as F32R and
    # the ACT copies that fill them do the rounding.
    GDT = F32R if gram_dtype == "f32r" else F32

    with TileContext(nc) as tc:
        with (
            tc.tile_pool(name="const", bufs=1) as constp,
            tc.tile_pool(name="xt", bufs=2) as xtp,
            tc.tile_pool(name="xn", bufs=10) as xnp,
            tc.tile_pool(name="v0p", bufs=4) as v0p,
            tc.tile_pool(name="v1p", bufs=3) as v1p,
            tc.tile_pool(name="mskp", bufs=3) as mskp,
            tc.tile_pool(name="tiny", bufs=4) as tiny,
            tc.tile_pool(name="outp", bufs=2) as outp,
            tc.tile_pool(name="ps_t", bufs=1, space="PSUM") as ps_t,
            tc.tile_pool(name="ps_b", bufs=1, space="PSUM") as ps_b,
            tc.tile_pool(name="ps_v", bufs=3, space="PSUM") as ps_v,
        ):
            # HWDGE serializes ~625ns per DMA: issue the startup-critical
            # loads first (x tiles 0-1 + the transpose identity), then the
            # rest of stroke 0, then everything that is needed later
            # x loads come in (128, 2*D) pair-tiles: one DMA covers two
            # 128-row tiles (contiguous in DRAM), halving the ~625ns/DMA
            # HWDGE serialization on the startup critical path
            x_all = x_in[:, :]

            def load_pair(s_, k_):
                xp = xnp.tile([128, 2 * D], F32, tag="xn2")
                # dest[p, t*D+d] = x[(s*8+k+t)*128 + p, d]
                nc.sync.dma_start(
                    out=xp,
                    in_=bass.AP(
                        x_all.tensor,
                        (s_ * 8 + k_) * 128 * D,
                        [[D, 128], [128 * D, 2], [1, D]],
                    ),
                )
                return xp

            xn0 = [load_pair(0, 0)]
            ident = constp.tile([128, 128], F32)
            nc.sync.dma_start(out=ident, in_=ident_in[:, :])
            xn0.append(load_pair(0, 2))
            identb = constp.tile([128, 128], BF16)
            nc.sync.dma_start(out=identb, in_=identb_in[:, :])
            negidentb = constp.tile([128, 128], BF16)
            nc.sync.dma_start(out=negidentb, in_=negidentb_in[:, :])
            xn0.append(load_pair(0, 4))
            xn0.append(load_pair(0, 6))
            ones1 = constp.tile([1, 128], F32)
            nc.sync.dma_start(out=ones1, in_=ones1_in[:, :])
            c128 = constp.tile([128, 8], U32)
            nc.sync.dma_start(out=c128, in_=c128_in[:, :])
            base_cols = constp.tile([128, n_strokes], F32)
            nc.sync.dma_start(out=base_cols, in_=base_in[:, :])
            ones1r = constp.tile([1, 128], GDT)
            nc.scalar.copy(ones1r, ones1)

            # row 1: centers repeated, precomputed on host (2D view keeps
            # each descriptor under the 64KB SDMA limit)
            nc.sync.dma_start(
                out=edges[1, :].rearrange("(a b) -> a b", b=2304),
                in_=row1_in[:].rearrange("(a b) -> a b", b=2304),
            )

            edges_r0 = edges[0, :].rearrange(
                "(s t p j) -> s p t j", s=n_strokes, t=8, p=128, j=KOUT
            )

            last_xt_copy = [None]
            last_brow = [None]
            tile_cp = []
            stroke_ctx = {}
            pair_xn = {}
            pair_ps = {}

            def prep_step(s, k, xn_pre=None):
                # ---- incremental stroke prep: one x-tile per call so the
                # ACT work (square + XT merge) spreads evenly instead of
                # bursting and starving the per-tile psum copies ----
                if k == 0:
                    xtm = xtp.tile([128, 2 * L], GDT, tag="xtm")
                    brow = tiny.tile([1, L], GDT, tag="brow")
                    outbuf = outp.tile([128, 8 * KOUT], U32, tag="outbuf")
                    # (1,512) so ps_b fits one psum bank: the bias row is
                    # built in two halves, reusing this slot
                    sqrow_ps = ps_b.tile([1, 512], F32, tag="sqps")
                    sqcols = tiny.tile([128, 8], F32, tag="sqcols")
                    stroke_ctx[s] = (xtm, brow, outbuf, sqrow_ps, sqcols)
                xtm, brow, outbuf, sqrow_ps, sqcols = stroke_ctx[s][:5]
                if k % 2 == 0:
                    pair_xn[s] = xn_pre if xn_pre is not None else load_pair(s, k)
                xn = pair_xn[s][:, (k % 2) * D : (k % 2 + 1) * D]
                sq_scr = xnp.tile([128, D], F32, tag="sqscr")
                sqcol = sqcols[:, k : k + 1]
                nc.scalar.activation(
                    sq_scr,
                    xn,
                    mybir.ActivationFunctionType.Square,
                    accum_out=sqcol,
                )
                # one (128,512) ps_t slot holds TWO x-tiles' transposes
                # (same 1-bank footprint), so one ACT merge covers both
                if k % 2 == 0:
                    ps = ps_t.tile([128, 512], F32, tag="tp")
                    pair_ps[s] = ps
                else:
                    ps = pair_ps[s]
                for c in range(2):
                    tr = nc.tensor.transpose(
                        ps[:, (k % 2) * 256 + c * 128 : (k % 2) * 256 + (c + 1) * 128],
                        xn[:, c * 128 : (c + 1) * 128],
                        ident,
                    )
                    if k % 2 == 0 and c == 0 and last_xt_copy[0] is not None:
                        # Tile misses the cross-engine WAR wait when this
                        # transpose reuses a psum slot an ACT copy is
                        # still reading; ACT is in-order, so depending on
                        # the most recent copy covers all prior ones.
                        add_dep_helper(
                            tr.ins, last_xt_copy[0].ins, reason="ps_t WAR"
                        )
                if k % 2 == 1:
                    # merge both tiles, both chunks: ps layout is
                    # [tileA c0 | tileA c1 | tileB c0 | tileB c1], dest
                    # cols c*1024+(k-1)*128 .. +256 per chunk
                    dst = bass.AP(
                        xtm.tensor,
                        xtm.offset + (k - 1) * 128,
                        [xtm.ap[0], [1024, 2], [128, 2], [1, 128]],
                    )
                    src = bass.AP(
                        ps.tensor,
                        ps.offset,
                        [ps.ap[0], [128, 2], [256, 2], [1, 128]],
                    )
                    last_xt_copy[0] = nc.scalar.copy(dst, src)
                # sqrow_ps[0, (k%4)*128+p] = sq[k*128+p]; the single
                # (1,512) psum slot serves both halves of the stroke
                trs = nc.tensor.transpose(
                    sqrow_ps[:, (k % 4) * 128 : (k % 4 + 1) * 128], sqcol, ident
                )
                if k % 4 == 0 and last_brow[0] is not None:
                    # ps_b slot reuse: WAR vs the previous half-row's ACT
                    # brow read (cross-engine psum WAR, Tile under-syncs)
                    add_dep_helper(
                        trs.ins, last_brow[0].ins, reason="ps_b WAR"
                    )
                if k % 4 == 3:
                    # bias half-row: -0.5*sq + SHIFT (rides a K=1 fp32r
                    # matmul into the V0 accumulation group)
                    h0 = (k // 4) * 512
                    last_brow[0] = nc.scalar.activation(
                        brow[:, h0 : h0 + 512], sqrow_ps,
                        mybir.ActivationFunctionType.Copy,
                        scale=-0.5, bias=SHIFT,
                    )


            # two-stage deferred pipeline: stage 1 (mask matmul + 2nd psum
            # copy) lands one tile after the Pool mask, stage 2 (round-2
            # max8 + max_index) one tile after that, so the cross-engine
            # zap chain never stalls the DVE stream
            pend1 = [None]
            pend2 = [None]

            def flush2():
                if pend2[0] is None:
                    return
                ps_, pt_, v0_, v1_, r12_ = pend2[0]
                pend2[0] = None
                outbuf_ = stroke_ctx[ps_][2]
                nc.vector.max(out=r12_[:, 8:16], in_=v1_)
                if pt_ == 0:
                    # centers: staging col 0 of each 9-block, from constant.
                    # Emitted here (not in prep) so the DVE stream never
                    # blocks on the c128 DMA at startup.
                    nc.vector.tensor_copy(
                        bass.AP(
                            outbuf_.tensor,
                            outbuf_.offset,
                            [outbuf_.ap[0], [KOUT, 8], [1, 1]],
                        ),
                        c128,
                    )
                # neighbor ids: odd ranks 1,3,..,15 as one strided view,
                # indices written straight into the staging tile (kept
                # values are bit-exact in both v0 and v1, so one
                # max_index over v0 serves both rounds)
                nc.vector.max_index(
                    bass.AP(
                        outbuf_.tensor,
                        outbuf_.offset + pt_ * KOUT + 1,
                        [outbuf_.ap[0], [1, 8]],
                    ),
                    bass.AP(
                        r12_.tensor,
                        r12_.offset + 1,
                        [r12_.ap[0], [2, 8]],
                    ),
                    v0_,
                )
                if pt_ == 7:
                    # ---- per-stroke epilogue: +global base, DMA out ----
                    outbuf_u = outp.tile([128, 8 * KOUT], U32, tag="outbuf_u")
                    nc.gpsimd.tensor_scalar(
                        out=outbuf_u,
                        in0=outbuf_,
                        scalar1=base_cols[:, ps_ : ps_ + 1],
                        scalar2=None,
                        op0=AluOpType.add,
                    )
                    nc.sync.dma_start(
                        out=edges_r0[ps_],
                        in_=outbuf_u.rearrange("p (t j) -> p t j", j=KOUT),
                    )

            def flush1():
                if pend1[0] is None:
                    return
                flush2()
                ps_, pt_, v0ps_, msk_, v0_, r12_ = pend1[0]
                pend1[0] = None
                # fold -1e6 * mask into the still-open psum accumulation:
                # kept scores stay bit-exact (v0 + 0), round-1 winners and
                # rank7 drop ~1e6 below everything
                for h in range(2):
                    nc.tensor.matmul(
                        v0ps_[:, h * 512 : (h + 1) * 512],
                        lhsT=negidentb,
                        rhs=msk_[:, h * 512 : (h + 1) * 512],
                        start=False,
                        stop=(h == 1),
                        skip_group_check=True,
                    )
                v1_ = v1p.tile([128, L], F32, tag="v1")
                cp2 = nc.scalar.copy(v1_, v0ps_)
                tile_cp.append(cp2)
                pend2[0] = (ps_, pt_, v0_, v1_, r12_)

            # stroke 0 x loads were queued before the constants above
            for k in range(8):
                prep_step(0, k, xn_pre=xn0[k // 2] if k % 2 == 0 else None)
            for s in range(n_strokes):
                xtm, brow, outbuf = stroke_ctx[s][:3]

                # ---- per row-tile: matmul + topk ----
                for t in range(8):
                    v0ps = ps_v.tile([128, L], F32, tag="v0ps")
                    for h in range(2):
                        for c in range(2):
                            mmg = nc.tensor.matmul(
                                v0ps[:, h * 512 : (h + 1) * 512],
                                lhsT=xtm[:, c * L + t * 128 : c * L + (t + 1) * 128],
                                rhs=xtm[:, c * L + h * 512 : c * L + (h + 1) * 512],
                                start=(c == 0),
                                stop=False,
                                skip_group_check=True,
                            )
                            if c == 0 and len(tile_cp) >= 2:
                                # explicit WAR: this psum slot was last read
                                # by the 2nd ACT copy three tiles ago; Tile
                                # under-syncs writes-after-cross-engine-psum-
                                # reads here.  ACT is in-order, so that copy
                                # covers all prior readers of the slot.
                                add_dep_helper(
                                    mmg.ins, tile_cp[-2].ins, reason="ps_v WAR"
                                )
                        # bias last: keeps brow (squares chain) off the
                        # critical path of the first tiles of a stroke
                        nc.tensor.matmul(
                            v0ps[:, h * 512 : (h + 1) * 512],
                            lhsT=ones1r,
                            rhs=brow[:, h * 512 : (h + 1) * 512],
                            start=False,
                            stop=False,
                            skip_group_check=True,
                        )
                    nc.tensor.matmul(
                        v0ps[:, t * 128 : (t + 1) * 128],
                        lhsT=negidentb,
                        rhs=identb,
                        start=False,
                        stop=False,
                        skip_group_check=True,
                    )

                    # deferred stages first: their dependencies are 1-2
                    # tiles old, so the in-order DVE/ACT streams never
                    # block on this tile's fresh chain
                    flush1()
                    # V0 psum -> sbuf on ACT (max8 needs SBUF operands)
                    v0 = v0p.tile([128, L], F32, tag="v0")
                    nc.scalar.copy(v0, v0ps)
                    # ranks 0-7 and 8-15 land adjacently in one tile
                    r12 = tiny.tile([128, 16], F32, tag="r12")
                    nc.vector.max(out=r12[:, 0:8], in_=v0)
                    # round-1 winner mask on Pool: msk = (v0 >= rank7),
                    # exact 0/1 in bf16 (is_ge is the per-partition-scalar
                    # opcode the Pool engine accepts; mod and
                    # scalar_tensor_tensor fail the ISA engine check)
                    msk = mskp.tile([128, L], BF16, tag="msk")
                    nc.gpsimd.tensor_scalar(
                        out=msk,
                        in0=v0,
                        scalar1=r12[:, 7:8],
                        scalar2=None,
                        op0=AluOpType.is_ge,
                    )
                    pend1[0] = (s, t, v0ps, msk, v0, r12)
                    if s + 1 < n_strokes:
                        # one next-stroke prep step per tile, emitted after
                        # the tile's topk ops so the prep's ACT work (square
                        # + XT merge) queues behind this tile's psum copies
                        prep_step(s + 1, t)
            flush1()
            flush2()

    if split_waits:
        _split_sync_waits(nc)
    return nc


_NC_CACHE = None


def _get_program():
    global _NC_CACHE
    if _NC_CACHE is None:
        _NC_CACHE = _build_program()
    return _NC_CACHE


def kernel(**inputs: np.ndarray) -> np.ndarray:
    x = np.ascontiguousarray(np.asarray(inputs["x"], dtype=np.float32))
    assert x.shape == (S * L, D), x.shape

    nc = _get_program()
    in_maps = _in_maps_for(x)
    res = run_bass_kernel_spmd(nc, in_maps, list(range(N_CORES)))
    out = np.concatenate(
        [res.results[c]["edges"] for c in range(N_CORES)], axis=1
    )
    return out.astype(np.int32)


def _in_maps_for(x, n_strokes=S_PER_CORE):
    centers = np.arange(S * L, dtype=np.uint32)
    row1_full = np.repeat(centers, KOUT)
    pts = n_strokes * L
    cols = pts * KOUT
    c128 = (np.arange(8, dtype=np.uint32)[None, :] * 128
            + np.arange(128, dtype=np.uint32)[:, None])
    in_maps = []
    for c in range(N_CORES):
        base = np.zeros((128, n_strokes), dtype=np.float32)
        base[:, :] = (c * pts
                      + np.arange(n_strokes, dtype=np.float32)[None, :] * L)
        in_maps.append(
            {
                "x_shard": np.ascontiguousarray(
                    x[c * pts : (c + 1) * pts, :]
                ),
                "row1_const": row1_full[c * cols : (c + 1) * cols],
                "base_cols": base,
                "ident_c": np.eye(128, dtype=np.float32),
                "identb_c": np.eye(128, dtype=ml_dtypes.bfloat16),
                "negidentb_c": (NEG_BIG * np.eye(128)).astype(ml_dtypes.bfloat16),
                "ones1_c": np.ones((1, 128), dtype=np.float32),
                "c128_c": c128,
            }
        )
    return in_maps


def _timed_runner(nc, in_maps, iters):
    """Median wall-clock ns per execution of the sharded NEFF."""
    import time

    import jax
    from jax.sharding import Mesh, NamedSharding, PartitionSpec

    try:
        from jax.experimental.shard_map import shard_map
    except ImportError:
        from jax.shard_map import shard_map
    from concourse import bass2jax as b2j

    b2j.install_neuronx_cc_hook()
    n_cores = len(in_maps)
    partition_name = nc.partition_id_tensor.name if nc.partition_id_tensor else None
    in_names, out_names, out_avals, zero_outs = [], [], [], []
    for alloc in nc.m.functions[0].allocations:
        if not isinstance(alloc, mybir.MemoryLocationSet):
            continue
        name = alloc.memorylocations[0].name
        if alloc.kind == "ExternalInput":
            if name != partition_name:
                in_names.append(name)
        elif alloc.kind == "ExternalOutput":
            shape = list(alloc.tensor_shape)
            np_dtype = mybir.dt.np(alloc.dtype)
            out_names.append(name)
            out_avals.append(jax.core.ShapedArray(tuple(shape), np_dtype))
            zero_outs.append(np.zeros(shape, np_dtype))
    n_params = len(in_names)
    n_outs = len(out_avals)
    all_names = in_names + out_names
    if partition_name is not None:
        all_names = all_names + [partition_name]

    def _body(*args):
        operands = list(args)
        if partition_name is not None:
            operands.append(b2j.partition_id_tensor())
        outs = b2j._bass_exec_p.bind(
            *operands,
            out_avals=tuple(out_avals),
            in_names=tuple(all_names),
            out_names=tuple(out_names),
            lowering_input_output_aliases=(),
            sim_require_finite=True,
            sim_require_nnan=True,
            nc=nc,
        )
        return tuple(outs)

    devices = jax.devices()[:n_cores]
    mesh = Mesh(np.asarray(devices), ("core",))
    spec = PartitionSpec("core")
    sharded = jax.jit(
        shard_map(
            _body,
            mesh=mesh,
            in_specs=(spec,) * (n_params + n_outs),
            out_specs=(spec,) * n_outs,
            check_rep=False,
        ),
        donate_argnums=tuple(range(n_params, n_params + n_outs)),
        keep_unused=True,
    )
    shd = NamedSharding(mesh, spec)
    concat_in = [
        jax.device_put(
            np.concatenate(
                [np.asarray(in_maps[c][nm]) for c in range(n_cores)], axis=0
            ),
            shd,
        )
        for nm in in_names
    ]
    concat_zeros = [
        np.zeros((n_cores * z.shape[0], *z.shape[1:]), z.dtype) for z in zero_outs
    ]

    def one_call():
        zs = [jax.device_put(z, shd) for z in concat_zeros]
        jax.block_until_ready(zs)
        t0 = time.perf_counter()
        out = sharded(*concat_in, *zs)
        jax.block_until_ready(out)
        return time.perf_counter() - t0

    one_call()  # warmup / compile
    one_call()
    times = [one_call() for _ in range(iters)]
    times.sort()
    return times[len(times) // 2] * 1e9


def measure_exec_ns(x, iters=30):
    x = np.ascontiguousarray(np.asarray(x, dtype=np.float32))
    return _timed_runner(_get_program(), _in_maps_for(x), iters)


_NULL_NC = None


def measure_null_ns(iters=30):
    """Dispatch overhead baseline: a bass program that just copies 128B."""
    global _NULL_NC
    if _NULL_NC is None:
        nc = bass.Bass(target_bir_lowering=False, trn_type="TRN2")
        a = nc.dram_tensor("a", [1, 32], F32, kind="ExternalInput")
        b = nc.dram_tensor("b", [1, 32], F32, kind="ExternalOutput")
        with TileContext(nc) as tc:
            with tc.tile_pool(name="p", bufs=1) as pool:
                t = pool.tile([1, 32], F32)
                nc.sync.dma_start(out=t, in_=a[:, :])
                nc.sync.dma_start(out=b[:, :], in_=t)
        _split_sync_waits(nc)
        _NULL_NC = nc
    in_maps = [{"a": np.zeros((1, 32), np.float32)} for _ in range(N_CORES)]
    return _timed_runner(_NULL_NC, in_maps, iters)


if __name__ == "__main__":
    rng = np.random.default_rng(0)
    x = rng.standard_normal((S * L, D), dtype=np.float32)
    e = kernel(x=x, batch=np.zeros(S * L, np.int64), sketch_stroke_num=np.full(S, L, np.int64))
    print(e.shape, e.dtype)
    print(e[:, :12])
